# revision 1
# baseline (speedup 1.0000x reference)
"""Trainium2 Bass kernel for nn_DepthwiseXCorr (SiamRPN-style depthwise
cross-correlation head), data-parallel over 8 NeuronCores.

Network (per sample):
  k = relu(bn(conv3x3(kernel)))      [256,7,7]   -> [256,5,5]
  s = relu(bn(conv3x3(search)))      [256,31,31] -> [256,29,29]
  feat = depthwise_xcorr(s, k)                   -> [256,25,25]
  h = relu(bn(conv1x1(feat)))                    -> [256,25,25]
  out = conv1x1(h) + b                           -> [256,25,25]

Mapping:
  - batch 128 sharded 16 samples/core across 8 cores (SPMD, no collectives)
  - BN folded into conv weights/biases on host
  - convs + 1x1 heads on the PE via fp32r matmuls (full-rate fp32),
    window shifts expressed directly in the rhs access patterns (no im2col)
  - depthwise xcorr on the Vector engine via fused per-partition-scalar
    multiply-accumulate (scalar_tensor_tensor), overlapped with the PE's
    conv of the next sample
  - PSUM evacuation fused with BN bias + ReLU on the Scalar engine
  - rows padded to even widths (fp32r requires even innermost AP counts)
"""
import os
import numpy as np

import bass_rust
import concourse.bass as bass
import concourse.mybir as mybir
import concourse.tile as tile
from concourse.bass_utils import run_bass_kernel_spmd

dt = mybir.dt
F32, F32R = dt.float32, dt.float32r
AF = mybir.ActivationFunctionType
ALU = mybir.AluOpType

N_CORES = 8
B, CIN, HID, OC = 128, 256, 256, 256
SPC = B // N_CORES  # samples per core (16)
EPS = 1e-5

# padded geometry
KW = 8         # kernel input row padded 7 -> 8
KFW = 6        # conv_kernel output row padded 5 -> 6
SW = 32        # search input row padded 31 -> 32
SFW = 30       # conv_search output row padded 29 -> 30
FW = 26        # xcorr/head output row padded 25 -> 26
KCOLS = SPC * 5 * KFW          # 480: conv_kernel psum free size (all samples)
SFY = [(0, 15), (15, 14)]      # conv_search output row halves (N=450/420)
HN = [(0, 326), (326, 324)]    # head matmul N splits of 650
PE_TAPS = int(os.environ.get("XC_PE_TAPS", "14"))  # of 25; rest fused-MAC on DVE
PE_TAPS_LAST = int(os.environ.get("XC_PE_TAPS_LAST", "21"))  # final sample (tail)
REPS = int(os.environ.get("KERNEL_REPS", "1"))  # device-program repeats (timing)
HWLOOP = int(os.environ.get("KERNEL_HWLOOP", "0"))  # For_i repeats (timing only)


def _split_multi_waits(nc):
    """This walrus build accepts at most ONE sync wait per instruction;
    Tile's wait assignment can attach several. Move extras onto prepended
    same-engine NoOps (engine streams are in-order, semantics identical)."""
    n = 0
    for fn in nc.m.functions:
        for bb in fn.blocks:
            changed = False
            out = []
            for inst in bb.instructions:
                si = inst.sync_info
                waits = list(si.on_wait) if si is not None and si.on_wait else []
                if len(waits) > 1:
                    for w in waits[:-1]:
                        no = bass_rust.InstNoOp(
                            name=nc.get_next_instruction_name(), ins=[], outs=[])
                        no.engine = inst.engine
                        no.sync_info = bass_rust.SyncInfo(on_wait=[w], on_update=[])
                        out.append(no)
                    inst.sync_info = bass_rust.SyncInfo(
                        on_wait=[waits[-1]],
                        on_update=list(si.on_update) if si.on_update else [])
                    changed = True
                    n += 1
                out.append(inst)
            if changed:
                bb.instructions = out
    return n


def _shifted(ap, extra_offset, free_dims):
    """Rebuild an SBUF tile AP with a free-dim window: keep partition dim,
    replace free dims, add an element offset."""
    return bass.AP(ap.tensor, ap.offset + extra_offset,
                   [list(ap.ap[0])] + [list(d) for d in free_dims])


def _build(n_samples=SPC, ablate=()):
    """ablate (devloop only): 'xcorr1' = 1-tap xcorr, 'convs1' = 1-tap
    conv_search, 'noheads' = skip head convs (wrong results, for timing)."""
    nc = bass.Bass(trn_type="TRN2", target_bir_lowering=False, debug=False)

    xk = nc.dram_tensor("xk", [2, 128, SPC * 7 * KW], F32R, kind="ExternalInput")
    xs = nc.dram_tensor("xs", [2, SPC, 128, 31 * SW], F32R, kind="ExternalInput")
    wkt = nc.dram_tensor("wkt", [2, 128, 9 * 256], F32R, kind="ExternalInput")
    wst = nc.dram_tensor("wst", [2, 128, 9 * 256], F32R, kind="ExternalInput")
    wh1t = nc.dram_tensor("wh1t", [2, 128, 256], F32R, kind="ExternalInput")
    wh2t = nc.dram_tensor("wh2t", [2, 128, 256], F32R, kind="ExternalInput")
    bias = nc.dram_tensor("bias", [2, 128, 4], F32, kind="ExternalInput")
    out = nc.dram_tensor("out", [2, SPC, 128, 625], F32, kind="ExternalOutput")

    with tile.TileContext(nc) as tc:
        with tc.tile_pool(name="w", bufs=1) as wp, \
             tc.tile_pool(name="xsp", bufs=3) as xsp, \
             tc.tile_pool(name="sfp", bufs=2) as sfp, \
             tc.tile_pool(name="accp", bufs=2) as accp, \
             tc.tile_pool(name="featp", bufs=2) as featp, \
             tc.tile_pool(name="hp", bufs=2) as hp, \
             tc.tile_pool(name="obp", bufs=3) as obp, \
             tc.tile_pool(name="psA", bufs=2, space="PSUM") as psA, \
             tc.tile_pool(name="psB", bufs=3, space="PSUM") as psB, \
             tc.tile_pool(name="psX", bufs=3, space="PSUM") as psX:

            # ---- resident weights / biases / kernel-branch input ----
            wk_sb, ws_sb, w1_sb, w2_sb, bias_sb, xk_sb = [], [], [], [], [], []
            for c in range(2):
                t = wp.tile([128, 9 * 256], F32R, tag=f"wk{c}")
                nc.sync.dma_start(out=t[:], in_=wkt.ap()[c])
                wk_sb.append(t)
                t = wp.tile([128, SPC * 7 * KW], F32R, tag=f"xk{c}")
                nc.sync.dma_start(out=t[:], in_=xk.ap()[c])
                xk_sb.append(t)
                t = wp.tile([128, 4], F32, tag=f"bias{c}")
                nc.sync.dma_start(out=t[:], in_=bias.ap()[c])
                bias_sb.append(t)
            for c in range(2):
                t = wp.tile([128, 9 * 256], F32R, tag=f"ws{c}")
                nc.sync.dma_start(out=t[:], in_=wst.ap()[c])
                ws_sb.append(t)
                t = wp.tile([128, 256], F32R, tag=f"w1{c}")
                nc.sync.dma_start(out=t[:], in_=wh1t.ap()[c])
                w1_sb.append(t)
                t = wp.tile([128, 256], F32R, tag=f"w2{c}")
                nc.sync.dma_start(out=t[:], in_=wh2t.ap()[c])
                w2_sb.append(t)

            iden = wp.tile([128, 128], F32, tag="iden")
            from concourse.masks import make_identity
            make_identity(nc, iden[:])

            # ---- conv_kernel: all samples batched in the free dim ----
            kf_sb = []
            for co in range(2):
                ps = psA.tile([128, KCOLS], F32, tag="ps")
                n_mm = 0
                for tap in range(9):
                    dy, dx = divmod(tap, 3)
                    for ci in range(2):
                        rhs = _shifted(xk_sb[ci][:], dy * KW + dx,
                                       [[7 * KW, n_samples], [KW, 5], [1, KFW]])
                        lhs = wk_sb[ci][:, tap * 256 + co * 128:tap * 256 + co * 128 + 128]
                        n_cols = n_samples * 5 * KFW
                        nc.tensor.matmul(out=ps[:, :n_cols], lhsT=lhs, rhs=rhs,
                                         start=(n_mm == 0), stop=(n_mm == 17))
                        n_mm += 1
                kf = wp.tile([128, KCOLS], F32, tag=f"kf{co}")
                nc.scalar.activation(out=kf[:], in_=ps[:], func=AF.Relu,
                                     bias=bias_sb[co][:, 0:1], scale=1.0)
                kf_sb.append(kf)

            def emit_conv_search(s):
                xw = []
                for ci in range(2):
                    t = xsp.tile([128, 31 * SW], F32R, tag=f"xs{ci}")
                    nc.sync.dma_start(out=t[:], in_=xs.ap()[ci, s])
                    xw.append(t)
                sf = []
                for co in range(2):
                    sft = sfp.tile([128, 29 * SFW], F32R, tag=f"sf{co}")
                    for (ys, nr), off in zip(SFY, (0, SFY[0][1] * SFW)):
                        ps = psA.tile([128, nr * SFW], F32, tag="ps")
                        n_mm = 0
                        cs_taps = 1 if 'convs1' in ablate else 9
                        for tap in range(cs_taps):
                            dy, dx = divmod(tap, 3)
                            for ci in range(2):
                                rhs = _shifted(xw[ci][:], (ys + dy) * SW + dx,
                                               [[SW, nr], [1, SFW]])
                                lhs = ws_sb[ci][:, tap * 256 + co * 128:
                                                tap * 256 + co * 128 + 128]
                                nc.tensor.matmul(out=ps[:], lhsT=lhs, rhs=rhs,
                                                 start=(n_mm == 0),
                                                 stop=(n_mm == cs_taps * 2 - 1))
                                n_mm += 1
                        nc.scalar.activation(out=sft[:, off:off + nr * SFW], in_=ps[:],
                                             func=AF.Relu, bias=bias_sb[co][:, 1:2],
                                             scale=1.0)
                    sf.append(sft)
                return sf

            def emit_xcorr(s, sf, pe_tap_count=None):
                # tap split: first (25 - PE_TAPS) taps as fused MACs on the
                # Vector engine; last PE_TAPS taps as diagonal-weight matmul
                # accumulation on the PE (diag built on ACT from identity),
                # folded into the DVE chain via its seed ops (in1 = PSUM).
                feat = []
                n_taps = 1 if 'xcorr1' in ablate else 25
                a = pe_tap_count if pe_tap_count is not None else PE_TAPS
                a = 0 if n_taps == 1 else min(a, n_taps - 1)
                dve_taps = list(range(n_taps - a))
                pe_taps = list(range(n_taps - a, n_taps))
                HV = [(0, 13), (13, 12)]  # xcorr row halves for PSUM banks

                def win_of(cc, t, rows=25, row0=0):
                    ty, tx = divmod(t, 5)
                    return _shifted(sf[cc][:], (row0 + ty) * SFW + tx,
                                    [[SFW, rows], [1, FW]])

                def kcol_of(cc, t):
                    ty, tx = divmod(t, 5)
                    c0 = s * 5 * KFW + ty * KFW + tx
                    return kf_sb[cc][:, c0:c0 + 1]

                for cc in range(2):
                    ft = featp.tile([128, 25 * FW], F32R, tag=f"ft{cc}")
                    view = [[FW, 25], [1, 25]]
                    ft_ap = _shifted(ft[:], 0, view)

                    pparts = None
                    if pe_taps:
                        dg = accp.tile([128, a * 128], F32R, tag=f"dg{cc}")
                        for i, t in enumerate(pe_taps):
                            nc.scalar.activation(
                                out=dg[:, i * 128:(i + 1) * 128], in_=iden[:],
                                func=AF.Copy, scale=kcol_of(cc, t))
                        pparts = []
                        for (r0, nr) in HV:
                            px = psX.tile([128, 13 * FW], F32, tag="px")
                            for i, t in enumerate(pe_taps):
                                nc.tensor.matmul(
                                    out=px[:, :nr * FW],
                                    lhsT=dg[:, i * 128:(i + 1) * 128],
                                    rhs=win_of(cc, t, rows=nr, row0=r0),
                                    start=(i == 0), stop=(i == len(pe_taps) - 1))
                            pparts.append(px)

                    a0 = accp.tile([128, 25 * FW], F32, tag=f"acc{cc}a")
                    a1 = accp.tile([128, 25 * FW], F32, tag=f"acc{cc}b")
                    aps = [_shifted(a0[:], 0, view), _shifted(a1[:], 0, view)]
                    cur = 0
                    sfF = [None, None]
                    for i, t in enumerate(dve_taps):
                        last = (i == len(dve_taps) - 1)
                        dst = ft_ap if last else aps[1 - cur if i else 0]
                        ksc = kcol_of(cc, t)
                        if i == 0 and pparts is not None:
                            # seed: acc = win*k + PE partial (two PSUM halves)
                            for (r0, nr), px in zip(HV, pparts):
                                dv = _shifted(a0[:], r0 * FW, [[FW, nr], [1, 25]])
                                if last:
                                    dv = _shifted(ft[:], r0 * FW, [[FW, nr], [1, 25]])
                                w = _shifted(sf[cc][:].bitcast(F32),
                                             (r0 + t // 5) * SFW + t % 5,
                                             [[SFW, nr], [1, 25]])
                                pxv = _shifted(px[:], 0, [[FW, nr], [1, 25]])
                                nc.vector.scalar_tensor_tensor(
                                    out=dv, in0=w, scalar=ksc, in1=pxv,
                                    op0=ALU.mult, op1=ALU.add)
                        elif i == 0:
                            nc.vector.tensor_scalar_mul(dst, _win_f32(sf, cc, t), ksc)
                        else:
                            nc.vector.scalar_tensor_tensor(
                                out=dst, in0=_win_f32(sf, cc, t), scalar=ksc,
                                in1=aps[cur], op0=ALU.mult, op1=ALU.add)
                            cur = 1 - cur
                    feat.append(ft)
                return feat

            def _win_f32(sf, cc, t):
                ty, tx = divmod(t, 5)
                return _shifted(sf[cc][:].bitcast(F32), ty * SFW + tx,
                                [[SFW, 25], [1, 25]])

            def emit_heads(s, feat):
                hs = []
                for co in range(2):
                    ht = hp.tile([128, 25 * FW], F32R, tag=f"h{co}")
                    for off, n in HN:
                        ps = psB.tile([128, HN[0][1]], F32, tag="hps")
                        for ci in range(2):
                            nc.tensor.matmul(
                                out=ps[:, :n],
                                lhsT=w1_sb[ci][:, co * 128:co * 128 + 128],
                                rhs=feat[ci][:, off:off + n],
                                start=(ci == 0), stop=(ci == 1))
                        nc.scalar.activation(out=ht[:, off:off + n], in_=ps[:, :n],
                                             func=AF.Relu, bias=bias_sb[co][:, 2:3],
                                             scale=1.0)
                    hs.append(ht)
                for co in range(2):
                    ob = obp.tile([128, 25 * FW], F32, tag=f"ob{co}")
                    for off, n in HN:
                        ps = psB.tile([128, HN[0][1]], F32, tag="hps")
                        for ci in range(2):
                            nc.tensor.matmul(
                                out=ps[:, :n],
                                lhsT=w2_sb[ci][:, co * 128:co * 128 + 128],
                                rhs=hs[ci][:, off:off + n],
                                start=(ci == 0), stop=(ci == 1))
                        nc.scalar.activation(out=ob[:, off:off + n], in_=ps[:, :n],
                                             func=AF.Identity, bias=bias_sb[co][:, 3:4],
                                             scale=1.0)
                    src = _shifted(ob[:], 0, [[FW, 25], [1, 25]])
                    nc.scalar.dma_start(out=out.ap()[co, s], in_=src)

            import contextlib
            loop_cm = (tc.For_i(0, HWLOOP, 1, name="hwrep") if HWLOOP
                       else contextlib.nullcontext())
            with loop_cm:
              for _rep in range(REPS):
                prev = None
                for s in range(n_samples):
                    sf = emit_conv_search(s)
                    feat = emit_xcorr(
                        s, sf,
                        pe_tap_count=(PE_TAPS_LAST if s == n_samples - 1 else None))
                    if 'noheads' in ablate:
                        for co in range(2):
                            src_ap = _shifted(feat[co][:].bitcast(F32), 0, [[FW, 25], [1, 25]])
                            nc.scalar.dma_start(out=out.ap()[co, s], in_=src_ap)
                        continue
                    if prev is not None:
                        emit_heads(prev[0], prev[1])
                    prev = (s, feat)
                if 'noheads' not in ablate:
                    emit_heads(prev[0], prev[1])
    _split_multi_waits(nc)
    return nc


_cache = {}


def _get_nc(n_samples=SPC):
    if n_samples not in _cache:
        _cache[n_samples] = _build(n_samples)
    return _cache[n_samples]


def _prep_host(inputs):
    """Fold BN, transpose/pack weights, pad inputs. Returns per-core in_maps."""
    f32 = np.float32
    kernel = np.asarray(inputs["kernel"], f32)
    search = np.asarray(inputs["search"], f32)

    def fold(w, g, b, m, v):
        inv = (g / np.sqrt(v + EPS)).astype(f32)
        return (w * inv[:, None, None, None]).astype(f32), (b - m * inv).astype(f32)

    wk_f, bk_f = fold(inputs["wk"], inputs["gk"], inputs["bk"], inputs["mk"], inputs["vk"])
    ws_f, bs_f = fold(inputs["ws"], inputs["gs"], inputs["bs"], inputs["ms"], inputs["vs"])
    wh1_f, bh1_f = fold(inputs["wh1"], inputs["gh"], inputs["bh"], inputs["mh"], inputs["vh"])
    wh2_f = np.asarray(inputs["wh2"], f32)[:, :, 0, 0]
    bh2_f = np.asarray(inputs["bh2"], f32)

    # lhsT packings: [ci_chunk, 128ci, tap*256+co]
    wkt = np.ascontiguousarray(
        np.transpose(wk_f, (1, 2, 3, 0)).reshape(2, 128, 9 * 256))
    wst = np.ascontiguousarray(
        np.transpose(ws_f, (1, 2, 3, 0)).reshape(2, 128, 9 * 256))
    wh1t = np.ascontiguousarray(wh1_f[:, :, 0, 0].T.reshape(2, 128, 256))
    wh2t = np.ascontiguousarray(wh2_f.T.reshape(2, 128, 256))
    biases = np.ascontiguousarray(
        np.stack([bk_f, bs_f, bh1_f, bh2_f], axis=1).reshape(2, 128, 4))

    # kernel input: [B,256,7,7] -> pad x to 8 -> per-core [2,128, s*56+y*8+x]
    kpad = np.zeros((B, CIN, 7, KW), f32)
    kpad[:, :, :, :7] = kernel
    # search input: [B,256,31,31] -> pad x to 32 -> per-core [2, s, 128, y*32+x]
    spad = np.zeros((B, CIN, 31, SW), f32)
    spad[:, :, :, :31] = search

    in_maps = []
    for core in range(N_CORES):
        sl = slice(core * SPC, (core + 1) * SPC)
        xk_c = np.ascontiguousarray(
            np.transpose(kpad[sl], (1, 0, 2, 3)).reshape(2, 128, SPC * 7 * KW))
        xs_c = np.ascontiguousarray(
            np.transpose(spad[sl], (1, 0, 2, 3)).reshape(2, 128, SPC, 31 * SW)
            .transpose(0, 2, 1, 3))
        in_maps.append({
            "xk": xk_c, "xs": xs_c, "wkt": wkt, "wst": wst,
            "wh1t": wh1t, "wh2t": wh2t, "bias": biases,
        })
    return in_maps


def kernel(_trace=False, **inputs):
    import time as _time
    nc = _get_nc()
    in_maps = _prep_host(inputs)
    _t0 = _time.time()
    res = run_bass_kernel_spmd(nc, in_maps, core_ids=list(range(N_CORES)),
                               trace=_trace)
    kernel.last_run_s = _time.time() - _t0
    outs = []
    for core in range(N_CORES):
        o = res.results[core]["out"]  # [2, SPC, 128, 625]
        outs.append(np.transpose(o, (1, 0, 2, 3)).reshape(SPC, OC, 25, 25))
    full = np.concatenate(outs, axis=0)
    if _trace:
        kernel.last_exec_time_ns = res.exec_time_ns
        kernel.last_trace = res.instructions_and_trace
    return full

